# revision 15
# baseline (speedup 1.0000x reference)
"""Multi-head attention (B=4, T=2048, D=1024, H=16, causal) on 8 TRN2 NeuronCores.

Sharding: tensor-parallel over heads — core c owns heads {2c, 2c+1}
(columns [128c, 128c+128) of the QKV projections, rows [128c, 128c+128) of Wo).
Each core computes q/k/v for its heads over all B*T tokens, causal attention,
and a partial output projection; the host sums the 8 partials and adds bo.

v2 layout/schedule:
- fp16 operands everywhere (PSUM accumulation stays f32); error budget is
  2e-2 and fp16 keeps us ~1e-3.
- "feature-major" activations ([feature, token]) so every matmul contracts
  over the partition dim; scores computed transposed (S_T[tk, tq]) so softmax
  needs no transpose before P@V; denominator comes from a ones-column in V.
- V transposed via DMA XBAR (dma_start(transpose=True)) — no PE transposes.
- Scores/exp/PV column-restricted to the causal region (c0 = 128*d).
- Attention is head-sequential per 512-token window; per batch b the j-loop
  is woven (in emission order) with the projections of batch b+1 and the
  output projection of batch b-1, keeping the Tensor engine dense while the
  Act engine runs the exps.
"""
import sys

sys.path.insert(0, "/opt/trn_rl_repo")

import numpy as np

import concourse.bacc as bacc
import concourse.tile as tile
from concourse import mybir
from concourse.bass_utils import run_bass_kernel_spmd

B, T, D, H, HD = 4, 2048, 1024, 16, 64
NCORES = 8
DPC = 128          # dout per core = 2 heads * 64
BT = B * T         # 8192
TW = 512           # tq window width
NKT = D // 128     # 8 contraction tiles for projections
NWIN = T // TW     # 4 tq windows per batch
HSTRIDE = 128           # per-head V_aug columns: 64 ones then 64 V; the PV lhsT is
                        # exactly [ones|V] so the denominator lands on PSUM rows 0:64
                        # and every engine AP stays 64-partition aligned
VSTRIDE = 2 * HSTRIDE   # 256: per-tk-tile V_aug columns
VONES = 64              # ones block width
VAUGW = (BT // 128) * VSTRIDE
SCALE = 1.0 / np.sqrt(HD)

f16 = mybir.dt.float16
f32 = mybir.dt.float32
MULT = mybir.AluOpType.mult

_cache = {}


def _build(with_bias: bool, debug: bool = False):
    nc = bacc.Bacc()
    xT = nc.dram_tensor("xT", [D, BT], f16, kind="ExternalInput")
    wq = nc.dram_tensor("wq", [D, DPC], f16, kind="ExternalInput")
    wk = nc.dram_tensor("wk", [D, DPC], f16, kind="ExternalInput")
    wv = nc.dram_tensor("wv", [D, DPC], f16, kind="ExternalInput")
    wo = nc.dram_tensor("wo", [DPC, D], f16, kind="ExternalInput")
    out = nc.dram_tensor("out", [D, BT], f16, kind="ExternalOutput")
    if debug:
        dbg_qT = nc.dram_tensor("dbg_qT", [128, BT], f32, kind="ExternalOutput")
        dbg_kT = nc.dram_tensor("dbg_kT", [128, BT], f32, kind="ExternalOutput")
        dbg_va = nc.dram_tensor("dbg_va", [128, (BT // 128) * VSTRIDE], f32, kind="ExternalOutput")
        dbg_oT = nc.dram_tensor("dbg_oT", [128, BT], f32, kind="ExternalOutput")
        dbg_ost = nc.dram_tensor("dbg_ost", [HSTRIDE, TW], f32, kind="ExternalOutput")
        dbg_rc = nc.dram_tensor("dbg_rc", [HSTRIDE, TW], f32, kind="ExternalOutput")
        dbg_p = nc.dram_tensor("dbg_p", [128, TW], f32, kind="ExternalOutput")
    if with_bias:
        bq = nc.dram_tensor("bq", [DPC, 1], f32, kind="ExternalInput")
        bk = nc.dram_tensor("bk", [DPC, 1], f32, kind="ExternalInput")
        bv = nc.dram_tensor("bv", [DPC, 1], f32, kind="ExternalInput")

    # tri[p, f] = 1.0 if f >= p else 0.0 (keep iff tq >= tk on the diagonal block)
    tri_np = np.zeros((128, 128), dtype=np.float16)
    p_idx = np.arange(128)[:, None]
    f_idx = np.arange(128)[None, :]
    tri_np[f_idx >= p_idx] = 1.0
    tri_dram = nc.inline_tensor(tri_np, name="tri")

    with tile.TileContext(nc) as tc:
        with (
            tc.tile_pool(name="pers", bufs=1) as pers,
            tc.tile_pool(name="xp", bufs=1) as xp,
            tc.tile_pool(name="vs", bufs=2) as vsp,
            tc.tile_pool(name="pp", bufs=1) as ppool,
            tc.tile_pool(name="nrm", bufs=1) as nrm,
            tc.tile_pool(name="outp", bufs=3) as outp,
            tc.tile_pool(name="ps", bufs=1, space="PSUM") as ps,
        ):
            wq_sb = pers.tile([128, D], f16, tag="wq")
            wk_sb = pers.tile([128, D], f16, tag="wk")
            wv_sb = pers.tile([128, D], f16, tag="wv")
            wo_sb = pers.tile([128, D], f16, tag="wo")
            qT = pers.tile([128, BT], f16, tag="qT")
            kT = pers.tile([128, BT], f16, tag="kT")
            oT = pers.tile([128, BT], f16, tag="oT")
            vaug = pers.tile([128, VAUGW], f16, tag="vaug")
            tri_sb = pers.tile([128, 128], f16, tag="tri")

            nc.sync.dma_start(tri_sb[:], tri_dram[:])
            # ones columns of V_aug (cols 0:32 of each head's 96-block; row 0 of
            # each PV output is then the softmax denominator at partition 0)
            vaug_ones = vaug[:, : (BT // 128) * VSTRIDE].rearrange(
                "p (t g w) -> p t g w", t=BT // 128, g=2
            )[:, :, :, 0:VONES]
            nc.gpsimd.memset(vaug_ones, 1.0)
            for kt in range(NKT):
                s = slice(kt * 128, kt * 128 + 128)
                nc.sync.dma_start(wq_sb[:, s], wq[s, :])
                nc.sync.dma_start(wk_sb[:, s], wk[s, :])
                nc.sync.dma_start(wv_sb[:, s], wv[s, :])
            nc.sync.dma_start(wo_sb[:], wo[:, :])
            if with_bias:
                bq_sb = pers.tile([128, 1], f32, tag="bq")
                bk_sb = pers.tile([128, 1], f32, tag="bk")
                bv_sb = pers.tile([128, 1], f32, tag="bv")
                nc.sync.dma_start(bq_sb[:], bq[:, :])
                nc.sync.dma_start(bk_sb[:], bk[:, :])
                nc.sync.dma_start(bv_sb[:], bv[:, :])

            # ---- stream generators; each yield = one weave step ----

            vaug4 = vaug[:, : (BT // 128) * VSTRIDE].rearrange(
                "p (t g w) -> p t g w", t=BT // 128, g=2
            )

            def proj_stream(b):
                """QKV projections for batch b's 2048 tokens (fp16, feature-major).
                All 16 x tiles are prefetched in one burst so the woven matmuls
                never wait on the sync queue."""
                xts = {}
                for gp2 in (0, 1):
                    gp = 2 * b + gp2
                    for kt in range(NKT):
                        x_t = xp.tile(
                            [128, 1024], f16, tag=f"x{gp2}_{kt}", name="x", bufs=2
                        )
                        nc.sync.dma_start(
                            x_t[:],
                            xT[kt * 128 : kt * 128 + 128, gp * 1024 : gp * 1024 + 1024],
                        )
                        xts[(gp2, kt)] = x_t
                yield
                for gp2 in (0, 1):
                    for half in (0, 1):
                        g = (2 * b + gp2) * 2 + half
                        tok = slice(g * TW, g * TW + TW)
                        q_ps = ps.tile([128, TW], f32, tag="q", name="q")
                        k_ps = ps.tile([128, TW], f32, tag="k", name="k")
                        v_ps = ps.tile([128, TW], f32, tag="v", name="v")
                        for kt in range(NKT):
                            s = slice(kt * 128, kt * 128 + 128)
                            st, sp = kt == 0, kt == NKT - 1
                            xs = xts[(gp2, kt)][:, half * TW : half * TW + TW]
                            nc.tensor.matmul(q_ps[:], wq_sb[:, s], xs, start=st, stop=sp)
                            nc.tensor.matmul(k_ps[:], wk_sb[:, s], xs, start=st, stop=sp)
                            nc.tensor.matmul(v_ps[:], wv_sb[:, s], xs, start=st, stop=sp)
                            yield
                        # drain psum: q/k to SBUF fp16 (DVE), then one batched
                        # DMA-XBAR transpose per head into vaug (token-major).
                        if with_bias:
                            nc.vector.tensor_scalar_add(qT[:, tok], q_ps[:], bq_sb[:])
                            nc.vector.tensor_scalar_add(kT[:, tok], k_ps[:], bk_sb[:])
                        else:
                            nc.vector.tensor_copy(qT[:, tok], q_ps[:])
                            nc.vector.tensor_copy(kT[:, tok], k_ps[:])
                        v_st = vsp.tile([128, TW], f16, tag="vst", bufs=3)
                        if with_bias:
                            nc.vector.tensor_scalar_add(v_st[:], v_ps[:], bv_sb[:])
                        else:
                            nc.vector.tensor_copy(v_st[:], v_ps[:])
                        t0 = g * 4  # first of 4 global tk tile indices
                        for hh in (0, 1):
                            nc.sync.dma_start(
                                vaug4[:, t0 : t0 + 4, hh, VONES:HSTRIDE],
                                v_st[hh * HD : hh * HD + HD, :],
                                transpose=True,
                            )
                        yield

            def attn_stream(b):
                """Causal attention for batch b, head-sequential per window."""
                tb = b * T
                for wi in range(NWIN):
                    win = slice(tb + wi * TW, tb + wi * TW + TW)
                    jmax = 4 * wi + 4
                    for h in (0, 1):
                        hs = slice(h * HD, h * HD + HD)
                        # full 128 partitions: lhsT is a 128-col vaug window so
                        # the compiler-automatic FWL (NumWeights==128) kicks in;
                        # rows 65.. accumulate garbage we never read.
                        o_ps = ps.tile([128, TW], f32, tag="o", name="o", bufs=2)
                        for j in range(jmax):
                            d = j - 4 * wi
                            c0 = 128 * d if d > 0 else 0
                            bj = slice(tb + j * 128, tb + j * 128 + 128)
                            s_pr = ps.tile([128, TW], f32, tag="s", name="s", bufs=3)
                            nc.tensor.matmul(
                                s_pr[:, c0:TW],
                                kT[hs, bj],
                                qT[hs, win][:, c0:TW],
                                start=True,
                                stop=True,
                            )
                            p_pr = ppool.tile([128, TW], f16, tag="p", bufs=5)
                            nc.scalar.activation(
                                p_pr[:, c0:TW],
                                s_pr[:, c0:TW],
                                mybir.ActivationFunctionType.Exp,
                                scale=float(SCALE),
                            )
                            if debug and b == 1 and wi == 2 and h == 0 and j == 5:
                                pdump = nrm.tile([128, TW], f32, tag="pd")
                                nc.vector.tensor_copy(pdump[:], p_pr[:])
                                nc.sync.dma_start(dbg_p[:], pdump[:])
                            if d >= 0:  # diagonal tile: zero strict lower triangle
                                nc.gpsimd.tensor_tensor(
                                    p_pr[:, c0 : c0 + 128],
                                    p_pr[:, c0 : c0 + 128],
                                    tri_sb[:],
                                    MULT,
                                )
                            vcol = ((tb // 128) + j) * VSTRIDE + h * HSTRIDE
                            nc.tensor.matmul(
                                o_ps[:, c0:TW],
                                vaug[:, vcol : vcol + 128],
                                p_pr[:, c0:TW],
                                start=(j == 0),
                                stop=(j == jmax - 1),
                            )
                            yield
                        # normalize: o_ps rows 0:32 are the denominator (ones
                        # block of V_aug), rows 32:96 the head output.
                        o_st = nrm.tile([HSTRIDE, TW], f32, tag="ost", bufs=3)
                        nc.vector.tensor_copy(o_st[:], o_ps[0:HSTRIDE, :])
                        bc = nrm.tile([HSTRIDE, TW], f32, tag="bc", bufs=2)
                        nc.gpsimd.partition_broadcast(bc[:], o_st[0:1, :])
                        rc = nrm.tile([HSTRIDE, TW], f32, tag="rc", bufs=2)
                        # full-tile recip at partition base 0 (the custom-DVE
                        # ucode misbehaves on nonzero partition bases)
                        nc.vector.reciprocal_approx_fast(out=rc[:], in_=bc[:])
                        nc.vector.tensor_tensor(
                            oT[hs, win], o_st[VONES:HSTRIDE, :], rc[VONES:HSTRIDE, :], MULT
                        )
                        if debug and b == 1 and wi == 2 and h == 0:
                            nc.sync.dma_start(dbg_ost[:], o_st[:])
                            nc.sync.dma_start(dbg_rc[:], rc[:])
                        yield

            def outproj_stream(b):
                """Partial output projection for batch b: out[:, b] = Wo^T oT."""
                tb = b * T
                for dm in range(NKT):
                    s = slice(dm * 128, dm * 128 + 128)
                    st2 = outp.tile([128, T], f16, tag="st", bufs=3)
                    for tp in range(2):
                        for i2 in range(2):
                            o4 = tp * 1024 + i2 * TW
                            pr = ps.tile([128, TW], f32, tag="s", name="pr", bufs=3)
                            nc.tensor.matmul(
                                pr[:], wo_sb[:, s], oT[:, tb + o4 : tb + o4 + TW],
                                start=True, stop=True,
                            )
                            nc.vector.tensor_copy(st2[:, o4 : o4 + TW], pr[:])
                        yield
                    nc.sync.dma_start(out[s, tb : tb + T], st2[:])

            def drain(stream):
                if stream is not None:
                    for _ in stream:
                        pass

            def weave(main, aux1, aux2, n_main=88, n_aux1=37, n_aux2=16):
                """Emit main stream; spread aux streams evenly between steps."""
                e1 = e2 = 0
                for i, _ in enumerate(main, start=1):
                    if aux1 is not None:
                        want = i * n_aux1 // n_main
                        while e1 < want and next(aux1, StopIteration) is not StopIteration:
                            e1 += 1
                    if aux2 is not None:
                        want = i * n_aux2 // n_main
                        while e2 < want and next(aux2, StopIteration) is not StopIteration:
                            e2 += 1
                drain(aux1)
                drain(aux2)

            # ---- schedule: proj(0); attn(b) ⨝ proj(b+1) ⨝ outproj(b-1); outproj(3)
            drain(proj_stream(0))
            for b in range(B):
                pj = proj_stream(b + 1) if b + 1 < B else None
                op = outproj_stream(b - 1) if b - 1 >= 0 else None
                weave(attn_stream(b), pj, op)
            drain(outproj_stream(B - 1))

            if debug:
                with tc.tile_pool(name="dbgp", bufs=2) as dbgp:
                    for tg in range(BT // TW):
                        tok = slice(tg * TW, tg * TW + TW)
                        for name, sbuf, dram in (
                            ("q", qT, dbg_qT), ("k", kT, dbg_kT), ("o", oT, dbg_oT)
                        ):
                            t = dbgp.tile([128, TW], f32, tag="d", name="d")
                            nc.vector.tensor_copy(t[:], sbuf[:, tok])
                            nc.sync.dma_start(dram[:, tok], t[:])
                    for c0 in range(0, (BT // 128) * VSTRIDE, 520):
                        w = min(520, (BT // 128) * VSTRIDE - c0)
                        t = dbgp.tile([128, 520], f32, tag="d", name="d")
                        nc.vector.tensor_copy(t[:, 0:w], vaug[:, c0 : c0 + w])
                        nc.sync.dma_start(dbg_va[:, c0 : c0 + w], t[:, 0:w])

    nc.compile()
    return nc


def _get_nc(with_bias: bool, debug: bool = False):
    key = (with_bias, debug)
    if key not in _cache:
        _cache[key] = _build(with_bias, debug)
    return _cache[key]


def _make_in_maps(x, Wq, bq, Wk, bk, Wv, bv, Wo, with_bias):
    xT = np.ascontiguousarray(x.reshape(BT, D).T.astype(np.float16))
    in_maps = []
    for c in range(NCORES):
        cs = slice(c * DPC, c * DPC + DPC)
        m = {
            "xT": xT,
            "wq": np.ascontiguousarray(Wq[:, cs].astype(np.float16)),
            "wk": np.ascontiguousarray(Wk[:, cs].astype(np.float16)),
            "wv": np.ascontiguousarray(Wv[:, cs].astype(np.float16)),
            "wo": np.ascontiguousarray(Wo[cs, :].astype(np.float16)),
        }
        if with_bias:
            m["bq"] = np.ascontiguousarray(bq[cs]).reshape(DPC, 1).astype(np.float32)
            m["bk"] = np.ascontiguousarray(bk[cs]).reshape(DPC, 1).astype(np.float32)
            m["bv"] = np.ascontiguousarray(bv[cs]).reshape(DPC, 1).astype(np.float32)
        in_maps.append(m)
    return in_maps


def _gather(res, bo):
    acc = np.zeros((D, BT), dtype=np.float32)
    for r in res.results:
        acc += r["out"].astype(np.float32)
    y = acc.T + bo[None, :]
    return np.ascontiguousarray(y.reshape(B, T, D), dtype=np.float32)


def kernel(x, Wq, bq, Wk, bk, Wv, bv, Wo, bo, _trace=False, _debug=False):
    x = np.asarray(x, dtype=np.float32)
    Wq, Wk, Wv, Wo = (np.asarray(w, dtype=np.float32) for w in (Wq, Wk, Wv, Wo))
    bq, bk, bv, bo = (np.asarray(b_, dtype=np.float32) for b_ in (bq, bk, bv, bo))

    with_bias = bool(np.any(bq != 0) or np.any(bk != 0) or np.any(bv != 0))
    nc = _get_nc(with_bias, _debug)
    in_maps = _make_in_maps(x, Wq, bq, Wk, bk, Wv, bv, Wo, with_bias)
    res = run_bass_kernel_spmd(
        nc, in_maps, core_ids=list(range(NCORES)), trace=_trace
    )
    y = _gather(res, bo)
    if _trace or _debug:
        return y, res
    return y


# revision 16
# speedup vs baseline: 2.1059x; 2.1059x over previous
"""Multi-head attention (B=4, T=2048, D=1024, H=16, causal) on 8 TRN2 NeuronCores.

Sharding: tensor-parallel over heads — core c owns heads {2c, 2c+1}
(columns [128c, 128c+128) of the QKV projections, rows [128c, 128c+128) of Wo).
Each core computes q/k/v for its heads over all B*T tokens, causal attention,
and a partial output projection; the host sums the 8 partials and adds bo.

v2 layout/schedule:
- fp16 operands everywhere (PSUM accumulation stays f32); error budget is
  2e-2 and fp16 keeps us ~1e-3.
- "feature-major" activations ([feature, token]) so every matmul contracts
  over the partition dim; scores computed transposed (S_T[tk, tq]) so softmax
  needs no transpose before P@V; denominator comes from a ones-column in V.
- V transposed via DMA XBAR (dma_start(transpose=True)) — no PE transposes.
- Scores/exp/PV column-restricted to the causal region (c0 = 128*d).
- Attention is head-sequential per 512-token window; per batch b the j-loop
  is woven (in emission order) with the projections of batch b+1 and the
  output projection of batch b-1, keeping the Tensor engine dense while the
  Act engine runs the exps.
"""
import sys

sys.path.insert(0, "/opt/trn_rl_repo")

import numpy as np

import concourse.bacc as bacc
import concourse.tile as tile
from concourse import mybir
from concourse.bass_utils import run_bass_kernel_spmd

B, T, D, H, HD = 4, 2048, 1024, 16, 64
NCORES = 8
DPC = 128          # dout per core = 2 heads * 64
BT = B * T         # 8192
TW = 512           # tq window width
NKT = D // 128     # 8 contraction tiles for projections
NWIN = T // TW     # 4 tq windows per batch
HSTRIDE = 128           # per-head V_aug columns: 64 ones then 64 V; the PV lhsT is
                        # exactly [ones|V] so the denominator lands on PSUM rows 0:64
                        # and every engine AP stays 64-partition aligned
VSTRIDE = 2 * HSTRIDE   # 256: per-tk-tile V_aug columns
VONES = 64              # ones block width
VAUGW = (BT // 128) * VSTRIDE
SCALE = 1.0 / np.sqrt(HD)

f16 = mybir.dt.float16
f32 = mybir.dt.float32
MULT = mybir.AluOpType.mult

_cache = {}


def _build(with_bias: bool, debug: bool = False):
    nc = bacc.Bacc()
    xT = nc.dram_tensor("xT", [D, BT], f16, kind="ExternalInput")
    wq = nc.dram_tensor("wq", [D, DPC], f16, kind="ExternalInput")
    wk = nc.dram_tensor("wk", [D, DPC], f16, kind="ExternalInput")
    wv = nc.dram_tensor("wv", [D, DPC], f16, kind="ExternalInput")
    wo = nc.dram_tensor("wo", [DPC, D], f16, kind="ExternalInput")
    out = nc.dram_tensor("out", [D, BT], f16, kind="ExternalOutput")
    if debug:
        dbg_qT = nc.dram_tensor("dbg_qT", [128, BT], f32, kind="ExternalOutput")
        dbg_kT = nc.dram_tensor("dbg_kT", [128, BT], f32, kind="ExternalOutput")
        dbg_va = nc.dram_tensor("dbg_va", [128, (BT // 128) * VSTRIDE], f32, kind="ExternalOutput")
        dbg_oT = nc.dram_tensor("dbg_oT", [128, BT], f32, kind="ExternalOutput")
        dbg_ost = nc.dram_tensor("dbg_ost", [HSTRIDE, TW], f32, kind="ExternalOutput")
        dbg_rc = nc.dram_tensor("dbg_rc", [HSTRIDE, TW], f32, kind="ExternalOutput")
        dbg_p = nc.dram_tensor("dbg_p", [128, TW], f32, kind="ExternalOutput")
    if with_bias:
        bq = nc.dram_tensor("bq", [DPC, 1], f32, kind="ExternalInput")
        bk = nc.dram_tensor("bk", [DPC, 1], f32, kind="ExternalInput")
        bv = nc.dram_tensor("bv", [DPC, 1], f32, kind="ExternalInput")

    # tri[p, f] = 1.0 if f >= p else 0.0 (keep iff tq >= tk on the diagonal block)
    tri_np = np.zeros((128, 128), dtype=np.float16)
    p_idx = np.arange(128)[:, None]
    f_idx = np.arange(128)[None, :]
    tri_np[f_idx >= p_idx] = 1.0
    tri_dram = nc.inline_tensor(tri_np, name="tri")

    with tile.TileContext(nc) as tc:
        with (
            tc.tile_pool(name="pers", bufs=1) as pers,
            tc.tile_pool(name="xp", bufs=1) as xp,
            tc.tile_pool(name="vs", bufs=2) as vsp,
            tc.tile_pool(name="pp", bufs=1) as ppool,
            tc.tile_pool(name="nrm", bufs=1) as nrm,
            tc.tile_pool(name="outp", bufs=3) as outp,
            tc.tile_pool(name="ps", bufs=1, space="PSUM") as ps,
        ):
            wq_sb = pers.tile([128, D], f16, tag="wq")
            wk_sb = pers.tile([128, D], f16, tag="wk")
            wv_sb = pers.tile([128, D], f16, tag="wv")
            wo_sb = pers.tile([128, D], f16, tag="wo")
            qT = pers.tile([128, BT], f16, tag="qT")
            kT = pers.tile([128, BT], f16, tag="kT")
            oT = pers.tile([128, BT], f16, tag="oT")
            vaug = pers.tile([128, VAUGW], f16, tag="vaug")
            tri_sb = pers.tile([128, 128], f16, tag="tri")

            nc.sync.dma_start(tri_sb[:], tri_dram[:])
            # ones columns of V_aug (cols 0:32 of each head's 96-block; row 0 of
            # each PV output is then the softmax denominator at partition 0)
            vaug_ones = vaug[:, : (BT // 128) * VSTRIDE].rearrange(
                "p (t g w) -> p t g w", t=BT // 128, g=2
            )[:, :, :, 0:VONES]
            nc.gpsimd.memset(vaug_ones, 1.0)
            for kt in range(NKT):
                s = slice(kt * 128, kt * 128 + 128)
                nc.sync.dma_start(wq_sb[:, s], wq[s, :])
                nc.sync.dma_start(wk_sb[:, s], wk[s, :])
                nc.sync.dma_start(wv_sb[:, s], wv[s, :])
            nc.sync.dma_start(wo_sb[:], wo[:, :])
            if with_bias:
                bq_sb = pers.tile([128, 1], f32, tag="bq")
                bk_sb = pers.tile([128, 1], f32, tag="bk")
                bv_sb = pers.tile([128, 1], f32, tag="bv")
                nc.sync.dma_start(bq_sb[:], bq[:, :])
                nc.sync.dma_start(bk_sb[:], bk[:, :])
                nc.sync.dma_start(bv_sb[:], bv[:, :])

            # ---- stream generators; each yield = one weave step ----

            vaug4 = vaug[:, : (BT // 128) * VSTRIDE].rearrange(
                "p (t g w) -> p t g w", t=BT // 128, g=2
            )

            def proj_stream(b):
                """QKV projections for batch b's 2048 tokens (fp16, feature-major).
                All 16 x tiles are prefetched in one burst so the woven matmuls
                never wait on the sync queue."""
                xts = {}
                for gp2 in (0, 1):
                    gp = 2 * b + gp2
                    for kt in range(NKT):
                        x_t = xp.tile(
                            [128, 1024], f16, tag=f"x{gp2}_{kt}", name="x", bufs=2
                        )
                        nc.sync.dma_start(
                            x_t[:],
                            xT[kt * 128 : kt * 128 + 128, gp * 1024 : gp * 1024 + 1024],
                        )
                        xts[(gp2, kt)] = x_t
                yield
                for gp2 in (0, 1):
                    for half in (0, 1):
                        g = (2 * b + gp2) * 2 + half
                        tok = slice(g * TW, g * TW + TW)
                        q_ps = ps.tile([128, TW], f32, tag="q", name="q")
                        k_ps = ps.tile([128, TW], f32, tag="k", name="k")
                        v_ps = ps.tile([128, TW], f32, tag="v", name="v")
                        for kt in range(NKT):
                            s = slice(kt * 128, kt * 128 + 128)
                            st, sp = kt == 0, kt == NKT - 1
                            xs = xts[(gp2, kt)][:, half * TW : half * TW + TW]
                            nc.tensor.matmul(q_ps[:], wq_sb[:, s], xs, start=st, stop=sp)
                            nc.tensor.matmul(k_ps[:], wk_sb[:, s], xs, start=st, stop=sp)
                            nc.tensor.matmul(v_ps[:], wv_sb[:, s], xs, start=st, stop=sp)
                            yield
                        # drain psum: q/k to SBUF fp16 (DVE), then one batched
                        # DMA-XBAR transpose per head into vaug (token-major).
                        if with_bias:
                            nc.scalar.add(qT[:, tok], q_ps[:], bq_sb[:])
                            nc.vector.tensor_scalar_add(kT[:, tok], k_ps[:], bk_sb[:])
                        else:
                            nc.scalar.copy(qT[:, tok], q_ps[:])
                            nc.vector.tensor_copy(kT[:, tok], k_ps[:])
                        v_st = vsp.tile([128, TW], f16, tag="vst", bufs=3)
                        if with_bias:
                            nc.vector.tensor_scalar_add(v_st[:], v_ps[:], bv_sb[:])
                        else:
                            nc.vector.tensor_copy(v_st[:], v_ps[:])
                        t0 = g * 4  # first of 4 global tk tile indices
                        for hh in (0, 1):
                            nc.sync.dma_start(
                                vaug4[:, t0 : t0 + 4, hh, VONES:HSTRIDE],
                                v_st[hh * HD : hh * HD + HD, :],
                                transpose=True,
                            )
                        yield

            def attn_stream(b):
                """Causal attention for batch b, head-sequential per window."""
                tb = b * T
                for wi in range(NWIN):
                    win = slice(tb + wi * TW, tb + wi * TW + TW)
                    jmax = 4 * wi + 4
                    for h in (0, 1):
                        hs = slice(h * HD, h * HD + HD)
                        # full 128 partitions: lhsT is a 128-col vaug window so
                        # the compiler-automatic FWL (NumWeights==128) kicks in;
                        # rows 65.. accumulate garbage we never read.
                        o_ps = ps.tile([128, TW], f32, tag="o", name="o", bufs=2)
                        for j in range(jmax):
                            d = j - 4 * wi
                            c0 = 128 * d if d > 0 else 0
                            bj = slice(tb + j * 128, tb + j * 128 + 128)
                            s_pr = ps.tile([128, TW], f32, tag="s", name="s", bufs=3)
                            nc.tensor.matmul(
                                s_pr[:, c0:TW],
                                kT[hs, bj],
                                qT[hs, win][:, c0:TW],
                                start=True,
                                stop=True,
                            )
                            p_pr = ppool.tile([128, TW], f16, tag="p", bufs=5)
                            nc.scalar.activation(
                                p_pr[:, c0:TW],
                                s_pr[:, c0:TW],
                                mybir.ActivationFunctionType.Exp,
                                scale=float(SCALE),
                            )
                            if debug and b == 1 and wi == 2 and h == 0 and j == 5:
                                pdump = nrm.tile([128, TW], f32, tag="pd")
                                nc.vector.tensor_copy(pdump[:], p_pr[:])
                                nc.sync.dma_start(dbg_p[:], pdump[:])
                            if d >= 0:  # diagonal tile: zero strict lower triangle
                                nc.vector.tensor_tensor(
                                    p_pr[:, c0 : c0 + 128],
                                    p_pr[:, c0 : c0 + 128],
                                    tri_sb[:],
                                    MULT,
                                )
                            vcol = ((tb // 128) + j) * VSTRIDE + h * HSTRIDE
                            nc.tensor.matmul(
                                o_ps[:, c0:TW],
                                vaug[:, vcol : vcol + 128],
                                p_pr[:, c0:TW],
                                start=(j == 0),
                                stop=(j == jmax - 1),
                            )
                            yield
                        # normalize: o_ps rows 0:32 are the denominator (ones
                        # block of V_aug), rows 32:96 the head output.
                        o_st = nrm.tile([HSTRIDE, TW], f32, tag="ost", bufs=3)
                        nc.vector.tensor_copy(o_st[:], o_ps[0:HSTRIDE, :])
                        bc = nrm.tile([HSTRIDE, TW], f32, tag="bc", bufs=2)
                        nc.gpsimd.partition_broadcast(bc[:], o_st[0:1, :])
                        rc = nrm.tile([HSTRIDE, TW], f32, tag="rc", bufs=2)
                        # full-tile recip at partition base 0 (the custom-DVE
                        # ucode misbehaves on nonzero partition bases)
                        nc.vector.reciprocal_approx_fast(out=rc[:], in_=bc[:])
                        nc.vector.tensor_tensor(
                            oT[hs, win], o_st[VONES:HSTRIDE, :], rc[VONES:HSTRIDE, :], MULT
                        )
                        if debug and b == 1 and wi == 2 and h == 0:
                            nc.sync.dma_start(dbg_ost[:], o_st[:])
                            nc.sync.dma_start(dbg_rc[:], rc[:])
                        yield

            def outproj_stream(b):
                """Partial output projection for batch b: out[:, b] = Wo^T oT."""
                tb = b * T
                for dm in range(NKT):
                    s = slice(dm * 128, dm * 128 + 128)
                    st2 = outp.tile([128, T], f16, tag="st", bufs=3)
                    for tp in range(2):
                        for i2 in range(2):
                            o4 = tp * 1024 + i2 * TW
                            pr = ps.tile([128, TW], f32, tag="s", name="pr", bufs=3)
                            nc.tensor.matmul(
                                pr[:], wo_sb[:, s], oT[:, tb + o4 : tb + o4 + TW],
                                start=True, stop=True,
                            )
                            if i2 == 0:
                                nc.scalar.copy(st2[:, o4 : o4 + TW], pr[:])
                            else:
                                nc.vector.tensor_copy(st2[:, o4 : o4 + TW], pr[:])
                        yield
                    nc.sync.dma_start(out[s, tb : tb + T], st2[:])

            def drain(stream):
                if stream is not None:
                    for _ in stream:
                        pass

            def weave(main, aux1, aux2, n_main=88, n_aux1=37, n_aux2=16):
                """Emit main stream; spread aux streams evenly between steps."""
                e1 = e2 = 0
                for i, _ in enumerate(main, start=1):
                    if aux1 is not None:
                        want = i * n_aux1 // n_main
                        while e1 < want and next(aux1, StopIteration) is not StopIteration:
                            e1 += 1
                    if aux2 is not None:
                        want = i * n_aux2 // n_main
                        while e2 < want and next(aux2, StopIteration) is not StopIteration:
                            e2 += 1
                drain(aux1)
                drain(aux2)

            # ---- schedule: proj(0); attn(b) ⨝ proj(b+1) ⨝ outproj(b-1); outproj(3)
            drain(proj_stream(0))
            for b in range(B):
                pj = proj_stream(b + 1) if b + 1 < B else None
                op = outproj_stream(b - 1) if b - 1 >= 0 else None
                weave(attn_stream(b), pj, op)
            drain(outproj_stream(B - 1))

            if debug:
                with tc.tile_pool(name="dbgp", bufs=2) as dbgp:
                    for tg in range(BT // TW):
                        tok = slice(tg * TW, tg * TW + TW)
                        for name, sbuf, dram in (
                            ("q", qT, dbg_qT), ("k", kT, dbg_kT), ("o", oT, dbg_oT)
                        ):
                            t = dbgp.tile([128, TW], f32, tag="d", name="d")
                            nc.vector.tensor_copy(t[:], sbuf[:, tok])
                            nc.sync.dma_start(dram[:, tok], t[:])
                    for c0 in range(0, (BT // 128) * VSTRIDE, 520):
                        w = min(520, (BT // 128) * VSTRIDE - c0)
                        t = dbgp.tile([128, 520], f32, tag="d", name="d")
                        nc.vector.tensor_copy(t[:, 0:w], vaug[:, c0 : c0 + w])
                        nc.sync.dma_start(dbg_va[:, c0 : c0 + w], t[:, 0:w])

    nc.compile()
    return nc


def _get_nc(with_bias: bool, debug: bool = False):
    key = (with_bias, debug)
    if key not in _cache:
        _cache[key] = _build(with_bias, debug)
    return _cache[key]


def _make_in_maps(x, Wq, bq, Wk, bk, Wv, bv, Wo, with_bias):
    xT = np.ascontiguousarray(x.reshape(BT, D).T.astype(np.float16))
    in_maps = []
    for c in range(NCORES):
        cs = slice(c * DPC, c * DPC + DPC)
        m = {
            "xT": xT,
            "wq": np.ascontiguousarray(Wq[:, cs].astype(np.float16)),
            "wk": np.ascontiguousarray(Wk[:, cs].astype(np.float16)),
            "wv": np.ascontiguousarray(Wv[:, cs].astype(np.float16)),
            "wo": np.ascontiguousarray(Wo[cs, :].astype(np.float16)),
        }
        if with_bias:
            m["bq"] = np.ascontiguousarray(bq[cs]).reshape(DPC, 1).astype(np.float32)
            m["bk"] = np.ascontiguousarray(bk[cs]).reshape(DPC, 1).astype(np.float32)
            m["bv"] = np.ascontiguousarray(bv[cs]).reshape(DPC, 1).astype(np.float32)
        in_maps.append(m)
    return in_maps


def _gather(res, bo):
    acc = np.zeros((D, BT), dtype=np.float32)
    for r in res.results:
        acc += r["out"].astype(np.float32)
    y = acc.T + bo[None, :]
    return np.ascontiguousarray(y.reshape(B, T, D), dtype=np.float32)


def kernel(x, Wq, bq, Wk, bk, Wv, bv, Wo, bo, _trace=False, _debug=False):
    x = np.asarray(x, dtype=np.float32)
    Wq, Wk, Wv, Wo = (np.asarray(w, dtype=np.float32) for w in (Wq, Wk, Wv, Wo))
    bq, bk, bv, bo = (np.asarray(b_, dtype=np.float32) for b_ in (bq, bk, bv, bo))

    with_bias = bool(np.any(bq != 0) or np.any(bk != 0) or np.any(bv != 0))
    nc = _get_nc(with_bias, _debug)
    in_maps = _make_in_maps(x, Wq, bq, Wk, bk, Wv, bv, Wo, with_bias)
    res = run_bass_kernel_spmd(
        nc, in_maps, core_ids=list(range(NCORES)), trace=_trace
    )
    y = _gather(res, bo)
    if _trace or _debug:
        return y, res
    return y


# revision 18
# speedup vs baseline: 2.1608x; 1.0261x over previous
"""Multi-head attention (B=4, T=2048, D=1024, H=16, causal) on 8 TRN2 NeuronCores.

Sharding: tensor-parallel over heads — core c owns heads {2c, 2c+1}
(columns [128c, 128c+128) of the QKV projections, rows [128c, 128c+128) of Wo).
Each core computes q/k/v for its heads over all B*T tokens, causal attention,
and a partial output projection; the host sums the 8 partials and adds bo.

Layout/schedule (v8):
- fp16 operands everywhere (PSUM accumulation stays f32); error budget is
  2e-2 and fp16 keeps us ~7e-4.
- "feature-major" activations ([feature, token]) so every matmul contracts
  over the partition dim; scores are computed transposed (S_T[tk, tq]) so the
  softmax needs no transpose before P@V.
- V_aug per tk-tile per head is [64 ones | 64 V] (token-major, via batched
  DMA-XBAR transposes): the PV lhsT is exactly that 128-column window, so PSUM
  rows 0:64 of the PV accumulation are the softmax denominator (at partition
  0, where gpsimd partition_broadcast can read it) and rows 64:128 the head
  output.  NumWeights==128 also enables the compiler's fast weight load.
- Scores/exp/PV are column-restricted to the causal region (c0 = 128*d).
- 1024-wide tq windows, heads sequential; per batch b the attention j-loop is
  woven (emission order = engine order) with the projections of batch b+1 and
  the output projection of batch b-1 / b, keeping the Tensor engine dense so
  the HAM power manager keeps the PE clock up.
- PSUM: proj accumulator [128,1024] (2 banks) + s_pr 2x[128,1024] (4) +
  o_ps [128,1024] (2) = 8 banks.
"""
import sys

sys.path.insert(0, "/opt/trn_rl_repo")

import numpy as np

import concourse.bacc as bacc
import concourse.tile as tile
from concourse import mybir
from concourse.bass_utils import run_bass_kernel_spmd

B, T, D, H, HD = 4, 2048, 1024, 16, 64
NCORES = 8
DPC = 128          # dout per core = 2 heads * 64
BT = B * T         # 8192
TW = 1024          # tq window width
NKT = D // 128     # 8 contraction tiles for projections
NWIN = T // TW     # 2 tq windows per batch
NJW = TW // 128    # 8 tk tiles per window span
HSTRIDE = 128           # per-head V_aug columns: 64 ones then 64 V
VSTRIDE = 2 * HSTRIDE   # 256: per-tk-tile V_aug columns
VONES = 64              # ones block width
VAUGW = (BT // 128) * VSTRIDE
SCALE = 1.0 / np.sqrt(HD)

f16 = mybir.dt.float16
f32 = mybir.dt.float32
MULT = mybir.AluOpType.mult

_cache = {}


def _build(with_bias: bool, debug: bool = False):
    nc = bacc.Bacc()
    xT = nc.dram_tensor("xT", [D, BT], f16, kind="ExternalInput")
    wq = nc.dram_tensor("wq", [D, DPC], f16, kind="ExternalInput")
    wk = nc.dram_tensor("wk", [D, DPC], f16, kind="ExternalInput")
    wv = nc.dram_tensor("wv", [D, DPC], f16, kind="ExternalInput")
    wo = nc.dram_tensor("wo", [DPC, D], f16, kind="ExternalInput")
    out = nc.dram_tensor("out", [D, BT], f16, kind="ExternalOutput")
    if debug:
        dbg_qT = nc.dram_tensor("dbg_qT", [128, BT], f32, kind="ExternalOutput")
        dbg_kT = nc.dram_tensor("dbg_kT", [128, BT], f32, kind="ExternalOutput")
        dbg_va = nc.dram_tensor("dbg_va", [128, VAUGW], f32, kind="ExternalOutput")
        dbg_oT = nc.dram_tensor("dbg_oT", [128, BT], f32, kind="ExternalOutput")
    if with_bias:
        bq = nc.dram_tensor("bq", [DPC, 1], f32, kind="ExternalInput")
        bk = nc.dram_tensor("bk", [DPC, 1], f32, kind="ExternalInput")
        bv = nc.dram_tensor("bv", [DPC, 1], f32, kind="ExternalInput")

    # tri[p, f] = 1.0 if f >= p else 0.0 (keep iff tq >= tk on the diagonal block)
    tri_np = np.zeros((128, 128), dtype=np.float16)
    p_idx = np.arange(128)[:, None]
    f_idx = np.arange(128)[None, :]
    tri_np[f_idx >= p_idx] = 1.0
    tri_dram = nc.inline_tensor(tri_np, name="tri")

    with tile.TileContext(nc) as tc:
        with (
            tc.tile_pool(name="pers", bufs=1) as pers,
            tc.tile_pool(name="xp", bufs=1) as xp,
            tc.tile_pool(name="vs", bufs=2) as vsp,
            tc.tile_pool(name="pp", bufs=1) as ppool,
            tc.tile_pool(name="nrm", bufs=1) as nrm,
            tc.tile_pool(name="outp", bufs=2) as outp,
            tc.tile_pool(name="ps", bufs=1, space="PSUM") as ps,
        ):
            wq_sb = pers.tile([128, D], f16, tag="wq")
            wk_sb = pers.tile([128, D], f16, tag="wk")
            wv_sb = pers.tile([128, D], f16, tag="wv")
            wo_sb = pers.tile([128, D], f16, tag="wo")
            qT = pers.tile([128, BT], f16, tag="qT")
            kT = pers.tile([128, BT], f16, tag="kT")
            oT = pers.tile([128, BT], f16, tag="oT")
            vaug = pers.tile([128, VAUGW], f16, tag="vaug")
            tri_sb = pers.tile([128, 128], f16, tag="tri")

            nc.sync.dma_start(tri_sb[:], tri_dram[:])
            vaug4 = vaug[:].rearrange("p (t g w) -> p t g w", t=BT // 128, g=2)
            nc.gpsimd.memset(vaug4[:, :, :, 0:VONES], 1.0)
            if with_bias:
                bq_sb = pers.tile([128, 1], f32, tag="bq")
                bk_sb = pers.tile([128, 1], f32, tag="bk")
                bv_sb = pers.tile([128, 1], f32, tag="bv")
                nc.sync.dma_start(bq_sb[:], bq[:, :])
                nc.sync.dma_start(bk_sb[:], bk[:, :])
                nc.sync.dma_start(bv_sb[:], bv[:, :])
            # weight loads interleaved with batch-0 x loads so the first proj
            # matmuls (kt=0) have their operands as early as possible
            xts0 = {}
            for kt in range(NKT):
                s = slice(kt * 128, kt * 128 + 128)
                nc.sync.dma_start(wq_sb[:, s], wq[s, :])
                nc.sync.dma_start(wk_sb[:, s], wk[s, :])
                nc.sync.dma_start(wv_sb[:, s], wv[s, :])
                for gp2 in (0, 1):
                    x_t = xp.tile(
                        [128, 1024], f16, tag=f"x{gp2}_{kt}", name="x", bufs=2
                    )
                    nc.sync.dma_start(x_t[:], xT[s, gp2 * 1024 : gp2 * 1024 + 1024])
                    xts0[(gp2, kt)] = x_t
            nc.sync.dma_start(wo_sb[:], wo[:, :])

            # ---- stream generators; each yield = one weave step ----

            def proj_stream(b, xts=None):
                """QKV projections for batch b's 2048 tokens (fp16, feature-major).
                One rotating [128,1024] PSUM accumulator; q, k, v sequentially
                per 1024-token supergroup; all x tiles prefetched in a burst."""
                if xts is None:
                    xts = {}
                    for gp2 in (0, 1):
                        gp = 2 * b + gp2
                        for kt in range(NKT):
                            s = slice(kt * 128, kt * 128 + 128)
                            x_t = xp.tile(
                                [128, 1024], f16, tag=f"x{gp2}_{kt}", name="x", bufs=2
                            )
                            nc.sync.dma_start(
                                x_t[:], xT[s, gp * 1024 : gp * 1024 + 1024]
                            )
                            xts[(gp2, kt)] = x_t
                    yield
                for gp2 in (0, 1):
                    gp = 2 * b + gp2
                    for w_sb, wkind in ((wq_sb, "q"), (wk_sb, "k"), (wv_sb, "v")):
                        v_st = None
                        if wkind == "v":
                            v_st = vsp.tile([128, 1024], f16, tag="vst", bufs=2)
                        for half in (0, 1):
                            tok = slice(
                                gp * 1024 + half * 512, gp * 1024 + half * 512 + 512
                            )
                            acc = ps.tile([128, 512], f32, tag="pj", name="pj", bufs=2)
                            for kt in range(NKT):
                                s = slice(kt * 128, kt * 128 + 128)
                                nc.tensor.matmul(
                                    acc[:],
                                    w_sb[:, s],
                                    xts[(gp2, kt)][:, half * 512 : half * 512 + 512],
                                    start=kt == 0, stop=kt == NKT - 1,
                                )
                                if kt % 2 == 1:
                                    yield
                            if wkind == "q":
                                if with_bias:
                                    nc.scalar.add(qT[:, tok], acc[:], bq_sb[:])
                                else:
                                    nc.scalar.copy(qT[:, tok], acc[:])
                            elif wkind == "k":
                                if with_bias:
                                    nc.vector.tensor_scalar_add(
                                        kT[:, tok], acc[:], bk_sb[:]
                                    )
                                else:
                                    nc.vector.tensor_copy(kT[:, tok], acc[:])
                            else:
                                hv = v_st[:, half * 512 : half * 512 + 512]
                                if with_bias:
                                    nc.vector.tensor_scalar_add(hv, acc[:], bv_sb[:])
                                else:
                                    nc.vector.tensor_copy(hv, acc[:])
                            yield
                        if wkind == "v":
                            t0 = gp * 8  # first of 8 global tk tile indices
                            for hh in (0, 1):
                                nc.sync.dma_start(
                                    vaug4[:, t0 : t0 + 8, hh, VONES:HSTRIDE],
                                    v_st[hh * HD : hh * HD + HD, :],
                                    transpose=True,
                                )

            def attn_stream(b):
                """Causal attention for batch b, head-sequential per window."""
                tb = b * T
                for wi in range(NWIN):
                    win = slice(tb + wi * TW, tb + wi * TW + TW)
                    jmax = NJW * (wi + 1)
                    for h in (0, 1):
                        hs = slice(h * HD, h * HD + HD)
                        o_ps = ps.tile([128, TW], f32, tag="o", name="o", bufs=1)
                        for j in range(jmax):
                            d = j - NJW * wi
                            c0 = 128 * d if d > 0 else 0
                            bj = slice(tb + j * 128, tb + j * 128 + 128)
                            s_pr = ps.tile([128, TW], f32, tag="s", name="s", bufs=2)
                            for cl, cr in ((c0, 512), (max(c0, 512), TW)):
                                if cl >= cr:
                                    continue
                                nc.tensor.matmul(
                                    s_pr[:, cl:cr],
                                    kT[hs, bj],
                                    qT[hs, win][:, cl:cr],
                                    start=True,
                                    stop=True,
                                )
                            p_pr = ppool.tile([128, TW], f16, tag="p", bufs=4)
                            nc.scalar.activation(
                                p_pr[:, c0:TW],
                                s_pr[:, c0:TW],
                                mybir.ActivationFunctionType.Exp,
                                scale=float(SCALE),
                            )
                            if d >= 0:  # diagonal tile: zero strict lower triangle
                                nc.vector.tensor_tensor(
                                    p_pr[:, c0 : c0 + 128],
                                    p_pr[:, c0 : c0 + 128],
                                    tri_sb[:],
                                    MULT,
                                )
                            vcol = ((tb // 128) + j) * VSTRIDE + h * HSTRIDE
                            for cl, cr in ((c0, 512), (max(c0, 512), TW)):
                                if cl >= cr:
                                    continue
                                nc.tensor.matmul(
                                    o_ps[:, cl:cr],
                                    vaug[:, vcol : vcol + HSTRIDE],
                                    p_pr[:, cl:cr],
                                    start=(j == 0),
                                    stop=(j == jmax - 1) and cr == TW,
                                )
                            yield
                        # normalize: o_ps rows 0:64 are the denominator (ones
                        # block of V_aug), rows 64:128 the head output.
                        o_st = nrm.tile([HSTRIDE, TW], f32, tag="ost", bufs=2)
                        nc.vector.tensor_copy(o_st[:], o_ps[:])
                        bc = nrm.tile([HSTRIDE, TW], f32, tag="bc", bufs=2)
                        nc.gpsimd.partition_broadcast(bc[:], o_st[0:1, :])
                        rc = nrm.tile([HSTRIDE, TW], f32, tag="rc", bufs=2)
                        # full-tile recip at partition base 0 (the custom-DVE
                        # ucode misbehaves on nonzero partition bases)
                        nc.vector.reciprocal_approx_fast(out=rc[:], in_=bc[:])
                        nc.vector.tensor_tensor(
                            oT[hs, win], o_st[VONES:HSTRIDE, :], rc[VONES:HSTRIDE, :], MULT
                        )
                        yield

            def outproj_stream(b, tps=(0, 1)):
                """Partial output projection for batch b: out[:, b] = Wo^T oT.
                outproj for token half tp only needs oT window wi=tp."""
                tb = b * T
                for tp in tps:
                    for dm in range(NKT):
                        s = slice(dm * 128, dm * 128 + 128)
                        o4 = tb + tp * 1024
                        st2 = outp.tile([128, 1024], f16, tag=f"st{dm % 2}", bufs=2)
                        for i2 in range(2):
                            pr = ps.tile([128, 512], f32, tag="pj", name="pr", bufs=2)
                            nc.tensor.matmul(
                                pr[:],
                                wo_sb[:, s],
                                oT[:, o4 + i2 * 512 : o4 + i2 * 512 + 512],
                                start=True, stop=True,
                            )
                            dst = st2[:, i2 * 512 : i2 * 512 + 512]
                            if dm % 2 == 0:
                                nc.scalar.copy(dst, pr[:])
                            else:
                                nc.vector.tensor_copy(dst, pr[:])
                        nc.sync.dma_start(out[s, o4 : o4 + 1024], st2[:])
                        yield

            def drain(stream):
                if stream is not None:
                    for _ in stream:
                        pass

            def adv(st):
                try:
                    next(st)
                    return True
                except StopIteration:
                    return False

            def weave(main, n_main, auxes):
                """auxes: list of [stream, n_steps, start_i, cap_in_loop]."""
                done = [0] * len(auxes)
                for i, _ in enumerate(main, start=1):
                    for a, (st, n, i0, cap) in enumerate(auxes):
                        if st is None or i <= i0:
                            continue
                        want = min(cap, (i - i0) * n // max(1, n_main - i0))
                        while done[a] < want and adv(st):
                            done[a] += 1
                for st, _, _, _ in auxes:
                    drain(st)

            # attn steps per batch: sum over wi of 2*(jmax+1)
            N_ATT = sum(2 * (NJW * (wi + 1) + 1) for wi in range(NWIN))  # 52
            N_PROJ = 1 + 2 * 3 * 2 * (NKT // 2 + 1)  # 61
            N_OP = 16

            # ---- schedule ----
            drain(proj_stream(0, xts0))
            for b in range(B):
                auxes = []
                if b + 1 < B:
                    auxes.append((proj_stream(b + 1), N_PROJ, 0, N_PROJ))
                if b - 1 >= 0:
                    auxes.append((outproj_stream(b - 1), N_OP, 0, N_OP))
                if b == B - 1:
                    # gated: tp0 of this batch's outproj after window 0 done
                    auxes.append((outproj_stream(b, (0,)), 8, 20, 8))
                weave(attn_stream(b), N_ATT, auxes)
            drain(outproj_stream(B - 1, (1,)))

            if debug:
                with tc.tile_pool(name="dbgp", bufs=2) as dbgp:
                    for tg in range(BT // 512):
                        tok = slice(tg * 512, tg * 512 + 512)
                        for name, sbuf, dram in (
                            ("q", qT, dbg_qT), ("k", kT, dbg_kT), ("o", oT, dbg_oT)
                        ):
                            t = dbgp.tile([128, 512], f32, tag="d", name="d")
                            nc.vector.tensor_copy(t[:], sbuf[:, tok])
                            nc.sync.dma_start(dram[:, tok], t[:])
                    for c0 in range(0, VAUGW, 512):
                        w = min(512, VAUGW - c0)
                        t = dbgp.tile([128, 512], f32, tag="d", name="d")
                        nc.vector.tensor_copy(t[:, 0:w], vaug[:, c0 : c0 + w])
                        nc.sync.dma_start(dbg_va[:, c0 : c0 + w], t[:, 0:w])

    nc.compile()
    return nc


def _get_nc(with_bias: bool, debug: bool = False):
    key = (with_bias, debug)
    if key not in _cache:
        _cache[key] = _build(with_bias, debug)
    return _cache[key]


def _make_in_maps(x, Wq, bq, Wk, bk, Wv, bv, Wo, with_bias):
    xT = np.ascontiguousarray(x.reshape(BT, D).T.astype(np.float16))
    in_maps = []
    for c in range(NCORES):
        cs = slice(c * DPC, c * DPC + DPC)
        m = {
            "xT": xT,
            "wq": np.ascontiguousarray(Wq[:, cs].astype(np.float16)),
            "wk": np.ascontiguousarray(Wk[:, cs].astype(np.float16)),
            "wv": np.ascontiguousarray(Wv[:, cs].astype(np.float16)),
            "wo": np.ascontiguousarray(Wo[cs, :].astype(np.float16)),
        }
        if with_bias:
            m["bq"] = np.ascontiguousarray(bq[cs]).reshape(DPC, 1).astype(np.float32)
            m["bk"] = np.ascontiguousarray(bk[cs]).reshape(DPC, 1).astype(np.float32)
            m["bv"] = np.ascontiguousarray(bv[cs]).reshape(DPC, 1).astype(np.float32)
        in_maps.append(m)
    return in_maps


def _gather(res, bo):
    acc = np.zeros((D, BT), dtype=np.float32)
    for r in res.results:
        acc += r["out"].astype(np.float32)
    y = acc.T + bo[None, :]
    return np.ascontiguousarray(y.reshape(B, T, D), dtype=np.float32)


def kernel(x, Wq, bq, Wk, bk, Wv, bv, Wo, bo, _trace=False, _debug=False):
    x = np.asarray(x, dtype=np.float32)
    Wq, Wk, Wv, Wo = (np.asarray(w, dtype=np.float32) for w in (Wq, Wk, Wv, Wo))
    bq, bk, bv, bo = (np.asarray(b_, dtype=np.float32) for b_ in (bq, bk, bv, bo))

    with_bias = bool(np.any(bq != 0) or np.any(bk != 0) or np.any(bv != 0))
    nc = _get_nc(with_bias, _debug)
    in_maps = _make_in_maps(x, Wq, bq, Wk, bk, Wv, bv, Wo, with_bias)
    res = run_bass_kernel_spmd(
        nc, in_maps, core_ids=list(range(NCORES)), trace=_trace
    )
    y = _gather(res, bo)
    if _trace or _debug:
        return y, res
    return y
